# revision 1
# baseline (speedup 1.0000x reference)
"""2-layer GCN on 8 Trainium2 NeuronCores (Bass/Tile SPMD kernel).

Math: reference computes, per layer,
    out = A_norm @ (in @ W) + b,   A_norm[d,s] = dis[d]*dis[s]*A_hat[d,s]
with A_hat = adjacency + self-loops, dis = 1/sqrt(deg).
We use associativity to aggregate first:
    out = dis ⊙ (A_hat @ (dis ⊙ in)) @ W + b
so the per-edge work is a pure gather+segment-sum of pre-scaled node
features (no per-edge multiplies).

Sharding: nodes are balanced across 8 cores x NSC superchunks of 512
"slots" each.  Edges are assigned to the core/superchunk of their dst
node, bucketed by src block (so gather indices fit in int16), sorted by
dst slot, and packed into 128-edge groups.  Each group is one
dma_gather (128 rows of 512B) + one one-hot matmul G.T @ S that
segment-sums the group into PSUM[128 feat, 512 slots].  Per-core group
slot offsets are runtime values (PE registers + dynamic slices) so a
single SPMD program serves all 8 cores.  Between layers, the scaled
hidden features are AllGathered so every core can gather any source
row locally.
"""

import os
import sys

import numpy as np

sys.path.insert(0, "/opt/trn_rl_repo")

P = 128          # partitions / group size
SC = 512         # slots per superchunk (= one PSUM bank of f32)
WMAX = 48        # max slot span of one 128-edge group
NB = 4           # src buckets (int16 gather indices need blocks <= 32768 rows)
NCORES = 8
F_IN = 128
F_HID = 128
F_OUT = 64
GCHUNK = 8       # groups per dma_gather (1024 idxs; >1024 crashes the Q7)


# ----------------------------------------------------------------- host prep
def _prep(x, edge_index):
    N, F = x.shape
    assert F == F_IN
    src0 = np.asarray(edge_index[0], dtype=np.int64)
    dst0 = np.asarray(edge_index[1], dtype=np.int64)
    E = src0.shape[0]

    deg = np.bincount(dst0, minlength=N).astype(np.float32) + 1.0
    dis = (1.0 / np.sqrt(deg)).astype(np.float32)
    xs = x.astype(np.float32) * dis[:, None]

    # --- node -> (core, superchunk, slot), balancing edge counts per bin
    NSC = int(np.ceil(N / (NCORES * SC)))
    nbins = NCORES * NSC
    R = NSC * SC                      # padded rows per core
    BLOCK = NCORES * R // NB
    assert BLOCK <= 32768 and NCORES * R % NB == 0

    order = np.argsort(-deg, kind="stable")
    k = np.arange(N)
    rnd = k // nbins                      # deal round
    col = k % nbins
    bin_of_sorted = np.where(rnd % 2 == 0, col, nbins - 1 - col)
    bin_id = np.empty(N, dtype=np.int64)
    bin_id[order] = bin_of_sorted
    pos_in_bin = np.empty(N, dtype=np.int64)
    pos_in_bin[order] = rnd

    npb = int(np.ceil(N / nbins))
    assert npb <= SC
    rng = np.random.default_rng(12345)
    perms = np.stack([rng.permutation(SC)[:npb] for _ in range(nbins)])
    slot = perms[bin_id, pos_in_bin]
    core = bin_id // NSC
    sc = bin_id % NSC
    node_row = core * R + sc * SC + slot          # padded global row id

    # --- messages (edges + self loops), sorted by (cell, slot)
    ms = node_row[np.concatenate([src0, np.arange(N)])]
    md = node_row[np.concatenate([dst0, np.arange(N)])]
    M = ms.shape[0]
    m_core = md // R
    m_sc = (md % R) // SC
    m_slot = md % SC
    m_bkt = ms // BLOCK
    cell = ((m_core * NSC) + m_sc) * NB + m_bkt
    key = cell * SC + m_slot
    o = np.argsort(key, kind="stable")
    ms_s, cell_s, slot_s = ms[o], cell[o], m_slot[o]

    ncells = NCORES * NSC * NB
    cell_starts = np.searchsorted(cell_s, np.arange(ncells))
    cell_ends = np.searchsorted(cell_s, np.arange(ncells) + 1)

    # --- pack each cell into groups of <=128 edges spanning < WMAX slots
    groups = [[] for _ in range(ncells)]   # (start, end, lo) into sorted arrays
    for c in range(ncells):
        s, e = int(cell_starts[c]), int(cell_ends[c])
        i = s
        sl = slot_s
        while i < e:
            j = min(i + P, int(np.searchsorted(sl[s:e], sl[i] + WMAX) + s))
            lo = min(int(sl[i]), SC - WMAX)
            assert int(sl[j - 1]) - lo < WMAX
            groups[c].append((i, j, lo))
            i = j
    NG = max(1, max(len(g) for g in groups))

    # --- per-core tables
    ncols = NSC * NB * NG
    idx_tab = np.zeros((NCORES, NSC * NB, NG * P), dtype=np.int16)
    srel_tab = np.full((NCORES, ncols, P), -1.0, dtype=np.float32)
    srel0_tab = np.full((NCORES, NSC, P), -1.0, dtype=np.float32)
    lo_tab = np.zeros((NCORES, ncols), dtype=np.int32)
    for c in range(ncells):
        co, rem = divmod(c, NSC * NB)
        scb = rem                    # (sc*NB + b) index
        sci, b = divmod(rem, NB)
        for g, (s, e, lo) in enumerate(groups[c]):
            n = e - s
            base = scb * NG + g
            idx_tab[co, scb, g * P : g * P + n] = (ms_s[s:e] - b * BLOCK).astype(
                np.int16
            )
            lo_tab[co, base] = lo
            if b == 0 and g == 0:
                srel0_tab[co, sci, :n] = slot_s[s:e].astype(np.float32)
            else:
                srel_tab[co, base, :n] = (slot_s[s:e] - lo).astype(np.float32)

    # wrap idx to [16, cols] then replicate to 128 partitions
    idx_wrapped = idx_tab.reshape(NCORES, NSC * NB, NG * P // 16, 16)
    idx_wrapped = np.transpose(idx_wrapped, (0, 1, 3, 2))  # [.., 16, NG*8]
    idx_wrapped = np.tile(idx_wrapped, (1, 1, 8, 1))       # [.., 128, NG*8]
    # final SBUF-layout table per core: [128, NSC*NB*NG*8]
    idx_sb = np.ascontiguousarray(
        np.transpose(idx_wrapped, (0, 2, 1, 3)).reshape(NCORES, P, -1)
    )
    srel_sb = np.ascontiguousarray(np.transpose(srel_tab, (0, 2, 1)))
    srel0_sb = np.ascontiguousarray(np.transpose(srel0_tab, (0, 2, 1)))

    # per-core dis column table [128, NT] (0 for empty slots)
    NT = R // P
    row_node = np.full(NCORES * R, -1, dtype=np.int64)
    row_node[node_row] = np.arange(N)
    dis_pad = np.zeros(NCORES * R, dtype=np.float32)
    dis_pad[node_row] = dis
    dis_sb = np.ascontiguousarray(
        dis_pad.reshape(NCORES, NT, P).transpose(0, 2, 1)
    )

    xs_pad = np.zeros((NCORES * R, F_IN), dtype=np.float32)
    xs_pad[node_row] = xs

    iota_w = np.broadcast_to(
        np.arange(WMAX, dtype=np.float32), (P, WMAX)
    ).copy()
    iota_sc = np.broadcast_to(np.arange(SC, dtype=np.float32), (P, SC)).copy()

    return dict(
        N=N, NSC=NSC, R=R, BLOCK=BLOCK, NG=NG, NT=NT,
        node_row=node_row, xs_pad=xs_pad,
        idx_sb=idx_sb, srel_sb=srel_sb, srel0_sb=srel0_sb,
        lo_tab=lo_tab, dis_sb=dis_sb, iota_w=iota_w, iota_sc=iota_sc,
    )


# ------------------------------------------------------------- bass program
def _build(pp):
    import concourse.bass as bass
    import concourse.bacc as bacc
    import concourse.mybir as mybir
    from concourse import tile

    f32 = mybir.dt.float32
    i16 = mybir.dt.int16
    i32 = mybir.dt.int32
    NSC, R, BLOCK, NG, NT = pp["NSC"], pp["R"], pp["BLOCK"], pp["NG"], pp["NT"]
    ncols = NSC * NB * NG

    # experiment knobs (benchmarking only; defaults = production)
    v_gtbufs = int(os.environ.get("GCN_GTBUFS", "3"))
    v_spf = os.environ.get("GCN_SPF", "1") == "1"   # single_packet
    v_nocc = os.environ.get("GCN_NOCC", "") == "1"  # skip collective
    v_gonly = os.environ.get("GCN_GONLY", "") == "1"  # gathers only
    v_qn = int(os.environ.get("GCN_QN", "1"))       # swdge queues used

    nc = bacc.Bacc(
        "TRN2", target_bir_lowering=False, debug=False, num_devices=NCORES
    )

    xs_d = nc.dram_tensor("xs", [NCORES * R, F_IN], f32, kind="ExternalInput")
    idx_d = nc.dram_tensor("idxt", [P, ncols * 8], i16, kind="ExternalInput")
    srel_d = nc.dram_tensor("srelt", [P, ncols], f32, kind="ExternalInput")
    srel0_d = nc.dram_tensor("srel0t", [P, NSC], f32, kind="ExternalInput")
    lo_d = nc.dram_tensor("lot", [1, ncols], i32, kind="ExternalInput")
    dis_d = nc.dram_tensor("dist", [P, NT], f32, kind="ExternalInput")
    iw_d = nc.dram_tensor("iotaw", [P, WMAX], f32, kind="ExternalInput")
    isc_d = nc.dram_tensor("iotasc", [P, SC], f32, kind="ExternalInput")
    W1_d = nc.dram_tensor("W1", [F_IN, F_HID], f32, kind="ExternalInput")
    b1_d = nc.dram_tensor("b1r", [P, F_HID], f32, kind="ExternalInput")
    W2_d = nc.dram_tensor("W2", [F_HID, F_OUT], f32, kind="ExternalInput")
    b2_d = nc.dram_tensor("b2r", [P, F_OUT], f32, kind="ExternalInput")
    out_d = nc.dram_tensor("out", [R, F_OUT], f32, kind="ExternalOutput")
    dbg = os.environ.get("GCN_DEBUG", "") == "1"
    if dbg:
        dbgv_d = nc.dram_tensor("dbgv", [P, R], f32, kind="ExternalOutput")
        dbgu_d = nc.dram_tensor("dbgu", [R, F_HID], f32, kind="ExternalOutput")
        dbgv2_d = nc.dram_tensor("dbgv2", [P, R], f32, kind="ExternalOutput")

    u2loc_d = nc.dram_tensor("u2loc", [R, F_HID], f32)
    u2full_d = nc.dram_tensor(
        "u2full", [NCORES * R, F_HID], f32, addr_space="Shared"
    )

    lo_regs = [
        nc.alloc_register(mybir.EngineType.PE, f"lo{i}") for i in range(NG)
    ]
    lo_vals = [
        bass.make_scalar_value(r, min_val=0, max_val=SC - WMAX)
        for r in lo_regs
    ]

    with tile.TileContext(nc) as tc:
        with (
            tc.tile_pool(name="const", bufs=1) as cpool,
            tc.tile_pool(name="vt", bufs=1) as vpool,
            tc.tile_pool(name="gin", bufs=v_gtbufs) as gpool,
            tc.tile_pool(name="smat", bufs=4) as spool,
            tc.tile_pool(name="s0mat", bufs=2) as s0pool,
            tc.tile_pool(name="bwork", bufs=3) as bpool,
            tc.tile_pool(name="psagg", bufs=2, space="PSUM") as pagg,
            tc.tile_pool(name="psmm", bufs=2, space="PSUM") as pmm,
        ):
            # ---- constants / tables resident in SBUF
            idx_sb = cpool.tile([P, ncols * 8], i16)
            srel_sb = cpool.tile([P, ncols], f32)
            srel0_sb = cpool.tile([P, NSC], f32)
            lo_sb = cpool.tile([1, ncols], i32)
            dis_sb = cpool.tile([P, NT], f32)
            iw_sb = cpool.tile([P, WMAX], f32)
            isc_sb = cpool.tile([P, SC], f32)
            W1_sb = cpool.tile([F_IN, F_HID], f32)
            b1_sb = cpool.tile([P, F_HID], f32)
            W2_sb = cpool.tile([F_HID, F_OUT], f32)
            b2_sb = cpool.tile([P, F_OUT], f32)
            for sb, d in [
                (idx_sb, idx_d), (srel_sb, srel_d), (srel0_sb, srel0_d),
                (lo_sb, lo_d), (dis_sb, dis_d), (iw_sb, iw_d),
                (isc_sb, isc_d), (W1_sb, W1_d), (b1_sb, b1_d),
                (W2_sb, W2_d), (b2_sb, b2_d),
            ]:
                nc.sync.dma_start(sb[:], d[:])

            vT = vpool.tile([P, R], f32)  # aggregated features^T (feat-major)

            def agg_layer(src_dram):
                """vT[:, :] = (A_hat @ src)^T for this core's R slots."""
                for sci in range(NSC):
                    ps = pagg.tile([P, SC], f32)
                    ngrp = NB * NG
                    gi = 0
                    for b in range(NB):
                        scb = sci * NB + b
                        gts = []
                        for g0 in range(0, NG, GCHUNK):
                            gn = min(GCHUNK, NG - g0)
                            gt = gpool.tile([P, GCHUNK, F_IN], f32, tag="gt")
                            gts.append(gt)
                            nc.gpsimd.dma_gather(
                                gt[:, :gn, :],
                                src_dram[b * BLOCK : (b + 1) * BLOCK, :],
                                idx_sb[
                                    :,
                                    (scb * NG + g0) * 8 : (scb * NG + g0 + gn)
                                    * 8,
                                ],
                                gn * P,
                                gn * P,
                                F_IN,
                                elem_step=F_IN,
                                single_packet=v_spf,
                                queue_num=(len(gts) + b) % v_qn,
                            )
                        if v_gonly:
                            nc.vector.tensor_copy(
                                vT[:, sci * SC : sci * SC + P],
                                gts[0][:, 0, :],
                            )
                            continue
                        nc.reg_load(
                            lo_regs, lo_sb[0:1, scb * NG : scb * NG + NG]
                        )
                        for g in range(NG):
                            col = scb * NG + g
                            gt = gts[g // GCHUNK]
                            gg = g % GCHUNK
                            if b == 0 and g == 0:
                                S0 = s0pool.tile([P, SC], f32, tag="s0")
                                nc.vector.tensor_scalar(
                                    S0[:],
                                    isc_sb[:],
                                    srel0_sb[:, sci : sci + 1],
                                    None,
                                    op0=mybir.AluOpType.is_equal,
                                )
                                nc.tensor.matmul(
                                    ps[:, :],
                                    gt[:, gg, :],
                                    S0[:],
                                    start=True,
                                    stop=(gi == ngrp - 1),
                                )
                            else:
                                S = spool.tile([P, WMAX], f32, tag="s")
                                nc.vector.tensor_scalar(
                                    S[:],
                                    iw_sb[:],
                                    srel_sb[:, col : col + 1],
                                    None,
                                    op0=mybir.AluOpType.is_equal,
                                )
                                nc.tensor.matmul(
                                    ps[:, bass.ds(lo_vals[g], WMAX)],
                                    gt[:, gg, :],
                                    S[:],
                                    start=False,
                                    stop=(gi == ngrp - 1),
                                )
                            gi += 1
                    if not v_gonly:
                        nc.vector.tensor_copy(
                            vT[:, sci * SC : (sci + 1) * SC], ps[:]
                        )

            # ---------------- layer 1
            agg_layer(xs_d)
            if dbg:
                nc.sync.dma_start(dbgv_d[:], vT[:])
            for t in range(NT):
                pb = pmm.tile([P, F_HID], f32, tag="pb")
                nc.tensor.matmul(
                    pb[:],
                    vT[:, t * P : (t + 1) * P],
                    W1_sb[:],
                    start=True,
                    stop=True,
                )
                u = bpool.tile([P, F_HID], f32, tag="u")
                nc.vector.tensor_scalar(
                    u[:], pb[:], dis_sb[:, t : t + 1], None,
                    op0=mybir.AluOpType.mult,
                )
                nc.vector.tensor_tensor(
                    u[:], u[:], b1_sb[:], op=mybir.AluOpType.add
                )
                nc.scalar.activation(
                    u[:], u[:], mybir.ActivationFunctionType.Relu
                )
                nc.vector.tensor_scalar(
                    u[:], u[:], dis_sb[:, t : t + 1], None,
                    op0=mybir.AluOpType.mult,
                )
                nc.sync.dma_start(u2loc_d[t * P : (t + 1) * P, :], u[:])

            # ---------------- exchange
            if v_nocc:
                nc.sync.dma_start(u2full_d[0:R, :], u2loc_d[:])
            else:
                nc.gpsimd.collective_compute(
                    "AllGather",
                    mybir.AluOpType.bypass,
                    replica_groups=[list(range(NCORES))],
                    ins=[u2loc_d[:]],
                    outs=[u2full_d[:]],
                )

            if dbg:
                nc.sync.dma_start(dbgu_d[:], u2loc_d[:])

            # ---------------- layer 2
            agg_layer(u2full_d)
            if dbg:
                nc.sync.dma_start(dbgv2_d[:], vT[:])
            for t in range(NT):
                pb = pmm.tile([P, F_OUT], f32, tag="pe")
                nc.tensor.matmul(
                    pb[:],
                    vT[:, t * P : (t + 1) * P],
                    W2_sb[:],
                    start=True,
                    stop=True,
                )
                u = bpool.tile([P, F_OUT], f32, tag="ue")
                nc.vector.tensor_scalar(
                    u[:], pb[:], dis_sb[:, t : t + 1], None,
                    op0=mybir.AluOpType.mult,
                )
                nc.vector.tensor_tensor(
                    u[:], u[:], b2_sb[:], op=mybir.AluOpType.add
                )
                nc.sync.dma_start(out_d[t * P : (t + 1) * P, :], u[:])

    nc.compile()
    return nc


# ------------------------------------------------------------------ driver
_CACHE = {}
TRACE = False
LAST_RESULTS = None


def kernel(x, edge_index, W1, b1, W2, b2):
    from concourse.bass_utils import run_bass_kernel_spmd

    x = np.asarray(x)
    edge_index = np.asarray(edge_index)
    W1 = np.asarray(W1, dtype=np.float32)
    b1 = np.asarray(b1, dtype=np.float32)
    W2 = np.asarray(W2, dtype=np.float32)
    b2 = np.asarray(b2, dtype=np.float32)

    pp = _prep(x, edge_index)
    key = (x.shape, edge_index.shape, pp["NG"])
    if key not in _CACHE:
        _CACHE[key] = _build(pp)
    nc = _CACHE[key]

    b1r = np.broadcast_to(b1, (P, F_HID)).copy()
    b2r = np.broadcast_to(b2, (P, F_OUT)).copy()
    in_maps = []
    for c in range(NCORES):
        in_maps.append(
            {
                "xs": pp["xs_pad"],
                "idxt": pp["idx_sb"][c],
                "srelt": pp["srel_sb"][c],
                "srel0t": pp["srel0_sb"][c],
                "lot": pp["lo_tab"][c][None, :],
                "dist": pp["dis_sb"][c],
                "iotaw": pp["iota_w"],
                "iotasc": pp["iota_sc"],
                "W1": W1,
                "b1r": b1r,
                "W2": W2,
                "b2r": b2r,
            }
        )
    res = run_bass_kernel_spmd(
        nc, in_maps, list(range(NCORES)), trace=TRACE
    )
    global LAST_RESULTS
    LAST_RESULTS = res
    outs = np.stack([np.asarray(r["out"]) for r in res.results])  # [C, R, FO]
    outs = outs.reshape(NCORES * pp["R"], F_OUT)
    return np.ascontiguousarray(outs[pp["node_row"]])

